# revision 6
# baseline (speedup 1.0000x reference)
"""Trainium2 Bass kernel for the Encoder-z0 ODE-ConvGRU problem.

Data-parallel over batch: 16 batch elements / 8 NeuronCores = 2 per core.
Per core, a 16-step backwards ConvGRU recurrence with an Euler ODE step,
followed by a 1x1-conv transform producing (mean_z0, std_z0).

Conv3x3 (SAME) is computed as 9 shifted matmuls accumulating in PSUM:
feature maps live in SBUF as zero-padded (34x34) images with channels on
partitions; offset (dy,dx) contributes lhsT[k].T @ shifted_view(rhs).

Partition layout (per batch element):
  LOW  = partitions 0-63   : h, h_ode, reset, cand, and all h-side elementwise
  HIGH = partitions 64-127 : x (in the 128-partition conv rhs buffers)
Weight rows are reordered on the host to match (h-part first, x-part second).
The `update` gate lands on HIGH partitions and is moved to LOW with an
SBUF->SBUF DMA (off the compute engines).

Matmuls run in float32r (full-rate PE streaming, ~1e-3 matmul accuracy);
elementwise runs in fp32.
"""

import os

import numpy as np

import concourse.bass as bass
import concourse.tile as tile
from concourse import bacc, mybir
from concourse import bass_utils

B, T, C, H, W = 16, 16, 64, 32, 32
HD = 64
NCORES = 8
BL = B // NCORES          # batch elements per core
P = H + 2                 # padded image edge (34)
NPIX = H * W              # 1024
MMD = mybir.dt.float32r   # matmul dtype
F32 = mybir.dt.float32

# stashed by kernel() for test harnesses
last_result = None


def _offsets():
    return [(dy, dx) for dy in range(3) for dx in range(3)]


def _build(dts, use_mask):
    """Build the SPMD program. dts: (T,) floats baked as immediates.
    Masks (when not all ones) are read from the per-core `ms` DRAM input."""
    nc = bacc.Bacc("TRN2", target_bir_lowering=False, debug=False,
                   num_devices=NCORES)

    xs_d = nc.dram_tensor("xs", [T, C, BL, H, W], MMD, kind="ExternalInput").ap()
    wg_d = nc.dram_tensor("wg", [2 * C, 9 * 2 * HD], MMD, kind="ExternalInput").ap()
    wc_d = nc.dram_tensor("wc", [2 * C, 9 * HD], MMD, kind="ExternalInput").ap()
    wo_d = nc.dram_tensor("wo", [HD, 9 * HD], MMD, kind="ExternalInput").ap()
    wt1_d = nc.dram_tensor("wt1", [HD, HD], MMD, kind="ExternalInput").ap()
    wt2_d = nc.dram_tensor("wt2", [HD, 2 * HD], MMD, kind="ExternalInput").ap()
    bg_d = nc.dram_tensor("bg", [2 * HD, 1], F32, kind="ExternalInput").ap()
    bc_d = nc.dram_tensor("bc", [HD, 1], F32, kind="ExternalInput").ap()
    bo_d = nc.dram_tensor("bo", [HD, 1], F32, kind="ExternalInput").ap()
    bt1_d = nc.dram_tensor("bt1", [HD, 1], F32, kind="ExternalInput").ap()
    bt2_d = nc.dram_tensor("bt2", [2 * HD, 1], F32, kind="ExternalInput").ap()
    if use_mask:
        msd = nc.dram_tensor("ms", [T, BL, HD, 1], F32, kind="ExternalInput").ap()
    mean_d = nc.dram_tensor("mean", [BL, HD, H, W], F32, kind="ExternalOutput").ap()
    std_d = nc.dram_tensor("std", [BL, HD, H, W], F32, kind="ExternalOutput").ap()

    AF = mybir.ActivationFunctionType
    offs = _offsets()

    with tile.TileContext(nc) as tc:
        with (
            tc.tile_pool(name="persist", bufs=1) as pp,
            tc.tile_pool(name="ew", bufs=3) as ew,
            tc.tile_pool(name="psum", bufs=4, space="PSUM") as psp,
        ):
            # ---- persistent state ----
            hbuf = [pp.tile([HD, P, P], MMD, name=f"hbuf{b}") for b in range(BL)]
            bufa = [pp.tile([2 * C, P, P], MMD, name=f"bufa{b}") for b in range(BL)]
            bufb = [pp.tile([2 * C, P, P], MMD, name=f"bufb{b}") for b in range(BL)]
            wg = pp.tile([2 * C, 9 * 2 * HD], MMD, name="wg")
            wc = pp.tile([2 * C, 9 * HD], MMD, name="wc")
            wo = pp.tile([HD, 9 * HD], MMD, name="wo")
            wt1 = pp.tile([HD, HD], MMD, name="wt1")
            wt2 = pp.tile([HD, 2 * HD], MMD, name="wt2")
            bg = pp.tile([2 * HD, 1], F32, name="bg")
            bc = pp.tile([HD, 1], F32, name="bc")
            bo = pp.tile([HD, 1], F32, name="bo")
            bt1 = pp.tile([HD, 1], F32, name="bt1")
            bt2 = pp.tile([2 * HD, 1], F32, name="bt2")

            for b in range(BL):
                nc.vector.memzero(hbuf[b][:])
                nc.vector.memzero(bufa[b][:])
                nc.vector.memzero(bufb[b][:])
            nc.sync.dma_start(wg[:], wg_d[:])
            nc.sync.dma_start(wc[:], wc_d[:])
            nc.sync.dma_start(wo[:], wo_d[:])
            nc.sync.dma_start(wt1[:], wt1_d[:])
            nc.sync.dma_start(wt2[:], wt2_d[:])
            nc.sync.dma_start(bg[:], bg_d[:])
            nc.sync.dma_start(bc[:], bc_d[:])
            nc.sync.dma_start(bo[:], bo_d[:])
            nc.sync.dma_start(bt1[:], bt1_d[:])
            nc.sync.dma_start(bt2[:], bt2_d[:])

            def conv(psum_t, wtile, rhs_buf, kdim, mdim):
                """3x3 conv: accumulate 9 shifted matmuls into psum_t
                (mdim, NPIX). rhs_buf: (kdim, P, P) padded."""
                for j in range(2):           # N = 1024 -> 2 x 512
                    r0 = 16 * j
                    for k, (dy, dx) in enumerate(offs):
                        rhs = rhs_buf[0:kdim, dy + r0:dy + r0 + 16, dx:dx + 32]
                        nc.tensor.matmul(
                            psum_t[:, 512 * j:512 * (j + 1)],
                            wtile[0:kdim, mdim * k:mdim * (k + 1)],
                            rhs,
                            start=(k == 0), stop=(k == 8),
                        )

            # interior views (strided) of padded buffers
            def intr(buf, p0, pn):
                return buf[p0:p0 + pn, 1:33, 1:33]

            # ---- recurrence ----
            for t in range(T):
                for b in range(BL):
                    # x(t) into HIGH halves of bufA/bufB (borders stay 0)
                    nc.sync.dma_start(intr(bufa[b], C, C), xs_d[t, :, b])
                    nc.sync.dma_start(intr(bufb[b], C, C), xs_d[t, :, b])

                    # ODE: h_ode = h + dt * tanh(conv(h) + bo)
                    ps_o = psp.tile([HD, NPIX], F32, tag="ps")
                    conv(ps_o, wo, hbuf[b], HD, HD)
                    t1 = ew.tile([HD, NPIX], F32, tag="t1")
                    nc.scalar.activation(t1[:], ps_o[:], AF.Tanh, bias=bo[:])
                    tmp = ew.tile([HD, NPIX], F32, tag="tmp")
                    nc.scalar.mul(tmp[:], t1[:], float(dts[t]))
                    tmp3 = tmp[:].rearrange("p (y x) -> p y x", y=H, x=W)
                    nc.vector.tensor_add(intr(bufa[b], 0, HD),
                                         intr(hbuf[b], 0, HD), tmp3)

                    # gates = sigmoid(conv([h_ode; x]) + bg)
                    ps_g = psp.tile([2 * HD, NPIX], F32, tag="ps")
                    conv(ps_g, wg, bufa[b], 2 * C, 2 * HD)
                    g = ew.tile([2 * HD, NPIX], F32, tag="g")
                    nc.scalar.activation(g[:], ps_g[:], AF.Sigmoid, bias=bg[:])

                    # rh = reset * h_ode  (LOW partitions)
                    g3 = g[0:HD, :].rearrange("p (y x) -> p y x", y=H, x=W)
                    nc.vector.tensor_mul(intr(bufb[b], 0, HD), g3,
                                         intr(bufa[b], 0, HD))
                    # update: HIGH -> LOW via SBUF-to-SBUF DMA
                    u = ew.tile([HD, NPIX], F32, tag="u")
                    nc.sync.dma_start(u[:], g[HD:2 * HD, :])

                    # cand = tanh(conv([rh; x]) + bc)
                    ps_c = psp.tile([HD, NPIX], F32, tag="ps")
                    conv(ps_c, wc, bufb[b], 2 * C, HD)
                    cand = ew.tile([HD, NPIX], F32, tag="cand")
                    nc.scalar.activation(cand[:], ps_c[:], AF.Tanh, bias=bc[:])

                    # h_next = h_ode + m*u*(cand - h_ode)
                    c3 = cand[:].rearrange("p (y x) -> p y x", y=H, x=W)
                    d = ew.tile([HD, NPIX], F32, tag="d")
                    d3 = d[:].rearrange("p (y x) -> p y x", y=H, x=W)
                    nc.vector.tensor_sub(d3, c3, intr(bufa[b], 0, HD))
                    ud = ew.tile([HD, NPIX], F32, tag="ud")
                    u3 = u[:].rearrange("p (y x) -> p y x", y=H, x=W)
                    nc.vector.tensor_mul(ud[:].rearrange("p (y x) -> p y x",
                                                         y=H, x=W), u3, d3)
                    if use_mask:
                        mt = ew.tile([HD, 1], F32, tag="mt")
                        nc.sync.dma_start(mt[:], msd[t, b])
                        ud2 = ew.tile([HD, NPIX], F32, tag="ud2")
                        nc.vector.tensor_single_scalar(
                            ud2[:], ud[:], mt[:], mybir.AluOpType.mult)
                        ud = ud2
                    ud3 = ud[:].rearrange("p (y x) -> p y x", y=H, x=W)
                    nc.vector.tensor_add(intr(hbuf[b], 0, HD),
                                         intr(bufa[b], 0, HD), ud3)

            # ---- transform_z0: conv1x1 -> ReLU -> conv1x1 ----
            for b in range(BL):
                ps1 = psp.tile([HD, NPIX], F32, tag="ps")
                for j in range(2):
                    rhs = hbuf[b][:, 1 + 16 * j:1 + 16 * j + 16, 1:33]
                    nc.tensor.matmul(ps1[:, 512 * j:512 * (j + 1)],
                                     wt1[:], rhs, start=True, stop=True)
                z = ew.tile([HD, NPIX], MMD, tag="z")
                nc.scalar.activation(z[:], ps1[:], AF.Relu, bias=bt1[:])
                ps2 = psp.tile([2 * HD, NPIX], F32, tag="ps")
                for j in range(2):
                    nc.tensor.matmul(ps2[:, 512 * j:512 * (j + 1)],
                                     wt2[:], z[:, 512 * j:512 * (j + 1)],
                                     start=True, stop=True)
                mso = ew.tile([2 * HD, NPIX], F32, tag="mso")
                nc.scalar.activation(mso[0:HD, :], ps2[0:HD, :], AF.Identity,
                                     bias=bt2[0:HD, :])
                nc.scalar.activation(mso[HD:2 * HD, :], ps2[HD:2 * HD, :],
                                     AF.Abs, bias=bt2[HD:2 * HD, :])
                nc.sync.dma_start(mean_d[b], mso[0:HD, :])
                nc.sync.dma_start(std_d[b], mso[HD:2 * HD, :])

    nc.compile()
    return nc


def kernel(input_tensor, time_steps, mask, w_gates, b_gates, w_can, b_can,
           w_ode, b_ode, w_t1, b_t1, w_t2, b_t2):
    global last_result
    input_tensor = np.asarray(input_tensor, np.float32)
    time_steps = np.asarray(time_steps, np.float32)
    mask = np.asarray(mask, np.float32)

    # host-side prep -------------------------------------------------
    # run_backwards: flip along T; layout (T, C, B, H, W) for clean DMA
    xs = np.ascontiguousarray(
        np.transpose(input_tensor[:, ::-1], (1, 2, 0, 3, 4)))  # (T,C,B,H,W)
    ts_rev = time_steps[::-1].astype(np.float64)
    dts = np.concatenate([[-0.01], ts_rev[1:] - ts_rev[:-1]]).astype(np.float32)
    ms_all = mask[:, ::-1].T.astype(np.float32)      # (T, B)
    use_mask = not np.all(ms_all == 1.0)

    def lhsT9(w):  # (O, I, 3, 3) -> (I, 9, O) with h-part rows first
        o, i = w.shape[0], w.shape[1]
        out = np.empty((i, 9, o), np.float32)
        for k, (dy, dx) in enumerate(_offsets()):
            if i == 2 * C:
                out[0:C, k] = w[:, C:2 * C, dy, dx].T   # h-part rows LOW
                out[C:2 * C, k] = w[:, 0:C, dy, dx].T   # x-part rows HIGH
            else:
                out[:, k] = w[:, :, dy, dx].T
        return np.ascontiguousarray(out.reshape(i, 9 * o))

    wg_h = lhsT9(np.asarray(w_gates, np.float32))
    wc_h = lhsT9(np.asarray(w_can, np.float32))
    wo_h = lhsT9(np.asarray(w_ode, np.float32))
    wt1_h = np.ascontiguousarray(np.asarray(w_t1, np.float32)[:, :, 0, 0].T)
    wt2_h = np.ascontiguousarray(np.asarray(w_t2, np.float32)[:, :, 0, 0].T)

    common = {
        "wg": wg_h, "wc": wc_h, "wo": wo_h, "wt1": wt1_h, "wt2": wt2_h,
        "bg": np.asarray(b_gates, np.float32).reshape(2 * HD, 1),
        "bc": np.asarray(b_can, np.float32).reshape(HD, 1),
        "bo": np.asarray(b_ode, np.float32).reshape(HD, 1),
        "bt1": np.asarray(b_t1, np.float32).reshape(HD, 1),
        "bt2": np.asarray(b_t2, np.float32).reshape(2 * HD, 1),
    }

    in_maps = []
    for core in range(NCORES):
        bsl = slice(core * BL, (core + 1) * BL)
        m = dict(common)
        m["xs"] = np.ascontiguousarray(xs[:, :, bsl])
        if use_mask:
            mcore = ms_all[:, bsl]                     # (T, BL)
            m["ms"] = np.ascontiguousarray(
                np.broadcast_to(mcore[:, :, None, None], (T, BL, HD, 1))
            ).astype(np.float32)
        in_maps.append(m)

    nc = _build(dts, use_mask)

    trace = bool(int(os.environ.get("KERNEL_TRACE", "0")))
    res = bass_utils.run_bass_kernel_spmd(
        nc, in_maps, core_ids=list(range(NCORES)), trace=trace)
    last_result = res

    mean = np.empty((B, HD, H, W), np.float32)
    std = np.empty((B, HD, H, W), np.float32)
    for core in range(NCORES):
        mean[core * BL:(core + 1) * BL] = res.results[core]["mean"]
        std[core * BL:(core + 1) * BL] = res.results[core]["std"]
    return mean, std


# revision 14
# speedup vs baseline: 1.2054x; 1.2054x over previous
"""Trainium2 Bass kernel for the Encoder-z0 ODE-ConvGRU problem.

Data-parallel over batch: 16 batch elements / 8 NeuronCores = 2 per core.
Per core, a 16-step backwards ConvGRU recurrence with an Euler ODE step,
followed by a 1x1-conv transform producing (mean_z0, std_z0).

Conv3x3 (SAME) is computed as 9 shifted matmuls accumulating in PSUM:
feature maps live in SBUF as zero-padded (34x34) images with channels on
partitions; offset (dy,dx) contributes lhsT[k].T @ shifted_view(rhs).

The two local batch elements are laid out on opposite partition halves
(b=0: 0-63, b=1: 64-127).  All M=64 convolutions (ODE, candidate halves,
first 1x1) are merged across the two batch elements into single full-array
K=128 x M=128 matmuls with block-diagonal weights, halving their PE time.
The candidate conv is further split into an x-part (independent of the
recurrent state - scheduled to fill the recurrence's serial tail) and an
rh-part, accumulating into the same PSUM bank.

dt = -1 steps fold the Euler scale into negated ODE weights (tanh is odd).
Matmuls run in float32r (full-rate PE streaming, ~1e-3 accuracy);
elementwise runs in fp32.
"""

import os

import numpy as np

import concourse.bass as bass
import concourse.tile as tile
from concourse import bacc, mybir
from concourse import bass_utils

B, T, C, H, W = 16, 16, 64, 32, 32
HD = 64
NCORES = 8
BL = B // NCORES          # batch elements per core
P = H + 2                 # padded image edge (34)
NPIX = H * W              # 1024
MMD = mybir.dt.float32r   # matmul dtype
F32 = mybir.dt.float32

last_result = None


def _offsets():
    return [(dy, dx) for dy in range(3) for dx in range(3)]


def _build(dts, use_mask):
    nc = bacc.Bacc("TRN2", target_bir_lowering=False, debug=False,
                   num_devices=NCORES)

    FC = 2 * C  # 128
    xs_d = nc.dram_tensor("xs", [T, C, BL, H, W], MMD, kind="ExternalInput").ap()
    wg_d = nc.dram_tensor("wg", [BL, FC, 9 * FC], MMD, kind="ExternalInput").ap()
    wcx_d = nc.dram_tensor("wcx", [FC, 9 * FC], MMD, kind="ExternalInput").ap()
    wch_d = nc.dram_tensor("wch", [FC, 9 * FC], MMD, kind="ExternalInput").ap()
    wo_d = nc.dram_tensor("wo", [FC, 2 * 9 * FC], MMD, kind="ExternalInput").ap()
    wt1_d = nc.dram_tensor("wt1", [FC, FC], MMD, kind="ExternalInput").ap()
    wt2_d = nc.dram_tensor("wt2", [FC, FC], MMD, kind="ExternalInput").ap()
    bg_d = nc.dram_tensor("bg", [BL, FC, 1], F32, kind="ExternalInput").ap()
    bc_d = nc.dram_tensor("bc", [FC, 1], F32, kind="ExternalInput").ap()
    bo_d = nc.dram_tensor("bo", [FC, 2], F32, kind="ExternalInput").ap()
    bt1_d = nc.dram_tensor("bt1", [FC, 1], F32, kind="ExternalInput").ap()
    bt2_d = nc.dram_tensor("bt2", [FC, 1], F32, kind="ExternalInput").ap()
    if use_mask:
        msd = nc.dram_tensor("ms", [T, BL, HD, 1], F32, kind="ExternalInput").ap()
    mean_d = nc.dram_tensor("mean", [BL, HD, H, W], F32, kind="ExternalOutput").ap()
    std_d = nc.dram_tensor("std", [BL, HD, H, W], F32, kind="ExternalOutput").ap()

    AF = mybir.ActivationFunctionType
    offs = _offsets()

    with tile.TileContext(nc) as tc:
        with (
            tc.tile_pool(name="persist", bufs=1) as pp,
            tc.tile_pool(name="ew", bufs=3) as ew,
            tc.tile_pool(name="psum", bufs=4, space="PSUM") as psp,
        ):
            # ---- persistent state ----
            hbuf = pp.tile([FC, P, P], MMD, name="hbuf")    # h: b0 low, b1 high
            xbuf = pp.tile([FC, P, P], MMD, name="xbuf")    # x: b0 low, b1 high
            rhbuf = pp.tile([FC, P, P], MMD, name="rhbuf")  # r*h_ode per half
            bufa = [pp.tile([FC, P, P], MMD, name=f"bufa{b}") for b in range(BL)]
            wg = [pp.tile([FC, 9 * FC], MMD, name=f"wg{b}") for b in range(BL)]
            wcx = pp.tile([FC, 9 * FC], MMD, name="wcx")
            wch = pp.tile([FC, 9 * FC], MMD, name="wch")
            wo = pp.tile([FC, 2 * 9 * FC], MMD, name="wo")
            wt1 = pp.tile([FC, FC], MMD, name="wt1")
            wt2 = pp.tile([FC, FC], MMD, name="wt2")
            bg = [pp.tile([FC, 1], F32, name=f"bg{b}") for b in range(BL)]
            bc = pp.tile([FC, 1], F32, name="bc")
            bo = pp.tile([FC, 2], F32, name="bo")           # [plain, negated]
            bt1 = pp.tile([FC, 1], F32, name="bt1")
            bt2 = pp.tile([FC, 1], F32, name="bt2")

            nc.vector.memzero(hbuf[:])
            nc.vector.memzero(xbuf[:])
            nc.vector.memzero(rhbuf[:])
            for b in range(BL):
                nc.vector.memzero(bufa[b][:])
                nc.sync.dma_start(wg[b][:], wg_d[b])
                nc.sync.dma_start(bg[b][:], bg_d[b])
            nc.sync.dma_start(wcx[:], wcx_d[:])
            nc.sync.dma_start(wch[:], wch_d[:])
            nc.sync.dma_start(wo[:], wo_d[:])
            nc.sync.dma_start(wt1[:], wt1_d[:])
            nc.sync.dma_start(wt2[:], wt2_d[:])
            nc.sync.dma_start(bc[:], bc_d[:])
            nc.sync.dma_start(bo[:], bo_d[:])
            nc.sync.dma_start(bt1[:], bt1_d[:])
            nc.sync.dma_start(bt2[:], bt2_d[:])

            def conv(psum_t, wtile, wcol0, rhs_buf, first, last):
                """Emit 18 full-width conv matmuls (9 offsets x 2 N-halves)."""
                for j in range(2):
                    r0 = 16 * j
                    for k, (dy, dx) in enumerate(offs):
                        nc.tensor.matmul(
                            psum_t[:, 512 * j:512 * (j + 1)],
                            wtile[:, wcol0 + FC * k:wcol0 + FC * (k + 1)],
                            rhs_buf[:, dy + r0:dy + r0 + 16, dx:dx + 32],
                            start=(first and k == 0), stop=(last and k == 8),
                            skip_group_check=True,
                        )

            def intr(buf, p0, pn):
                return buf[p0:p0 + pn, 1:33, 1:33]

            def r3(ap):
                return ap.rearrange("p (y x) -> p y x", y=H, x=W)

            def load_x(t):
                nc.sync.dma_start(intr(xbuf, 0, C), xs_d[t, :, 0])
                nc.sync.dma_start(intr(xbuf, C, C), xs_d[t, :, 1])
                nc.sync.dma_start(intr(bufa[0], C, C), xs_d[t, :, 0])
                nc.sync.dma_start(intr(bufa[1], 0, C), xs_d[t, :, 1])

            load_x(0)

            for t in range(T):
                # --- candidate conv, x part (independent of h) ---
                ps_c = psp.tile([FC, NPIX], F32, tag="ps", name="ps_c")
                conv(ps_c, wcx, 0, xbuf, True, False)

                # --- ODE step, both batch halves (block-diag weights) ---
                wcol = 9 * FC if dts[t] == -1.0 else 0
                ps_o = psp.tile([FC, NPIX], F32, tag="ps", name="ps_o")
                conv(ps_o, wo, wcol, hbuf, True, True)
                neg = 1 if dts[t] == -1.0 else 0
                t1 = ew.tile([FC, NPIX], F32, tag="t1")
                for b in range(BL):
                    ph = HD * b
                    nc.scalar.activation(t1[ph:ph + HD, :], ps_o[ph:ph + HD, :],
                                         AF.Tanh, bias=bo[ph:ph + HD, neg:neg + 1])
                src = t1
                if dts[t] not in (1.0, -1.0):
                    tmp = ew.tile([FC, NPIX], F32, tag="tmp")
                    nc.scalar.mul(tmp[:], t1[:], float(dts[t]))
                    src = tmp
                for b in range(BL):
                    ph = HD * b
                    nc.vector.tensor_add(intr(bufa[b], ph, HD),
                                         intr(hbuf, ph, HD),
                                         r3(src[ph:ph + HD, :]))

                # --- gates convs (per b, full K=M=128) + sigmoid, rh, u ---
                u = ew.tile([FC, NPIX], F32, tag="u")
                g = [None, None]
                for b in range(BL):
                    ph, px = HD * b, HD * (1 - b)
                    ps_g = psp.tile([FC, NPIX], F32, tag="ps", name="ps_g")
                    conv(ps_g, wg[b], 0, bufa[b], True, True)
                    gt = ew.tile([FC, NPIX], F32, tag=f"g{b}")
                    nc.scalar.activation(gt[:], ps_g[:], AF.Sigmoid, bias=bg[b][:])
                    g[b] = gt
                    nc.gpsimd.tensor_mul(intr(rhbuf, ph, HD),
                                         r3(gt[ph:ph + HD, :]),
                                         intr(bufa[b], ph, HD))
                    nc.sync.dma_start(u[ph:ph + HD, :], gt[px:px + HD, :])

                # --- candidate conv, rh part (accumulates into ps_c) ---
                conv(ps_c, wch, 0, rhbuf, False, True)

                # --- cand + combine per b ---
                cand = ew.tile([FC, NPIX], F32, tag="cand")
                for b in range(BL):
                    ph = HD * b
                    nc.scalar.activation(cand[ph:ph + HD, :], ps_c[ph:ph + HD, :],
                                         AF.Tanh, bias=bc[ph:ph + HD])
                d = ew.tile([FC, NPIX], F32, tag="d")
                ud = ew.tile([FC, NPIX], F32, tag="ud")
                for b in range(BL):
                    ph = HD * b
                    nc.gpsimd.tensor_sub(r3(d[ph:ph + HD, :]),
                                         r3(cand[ph:ph + HD, :]),
                                         intr(bufa[b], ph, HD))
                    nc.vector.tensor_mul(ud[ph:ph + HD, :], u[ph:ph + HD, :],
                                         d[ph:ph + HD, :])
                    uds = ud
                    if use_mask:
                        mt = ew.tile([FC, 1], F32, tag="mt")
                        nc.sync.dma_start(mt[ph:ph + HD, :], msd[t, b])
                        ud2 = ew.tile([FC, NPIX], F32, tag="ud2")
                        nc.vector.tensor_single_scalar(
                            ud2[ph:ph + HD, :], ud[ph:ph + HD, :],
                            mt[ph:ph + HD, :], mybir.AluOpType.mult)
                        uds = ud2
                    nc.vector.tensor_add(intr(hbuf, ph, HD),
                                         intr(bufa[b], ph, HD),
                                         r3(uds[ph:ph + HD, :]))

                if t + 1 < T:
                    load_x(t + 1)

            # ---- transform_z0: conv1x1 -> ReLU -> conv1x1 --------------
            ps1 = psp.tile([FC, NPIX], F32, tag="ps", name="ps1")
            for j in range(2):
                nc.tensor.matmul(ps1[:, 512 * j:512 * (j + 1)], wt1[:],
                                 hbuf[:, 1 + 16 * j:17 + 16 * j, 1:33],
                                 start=True, stop=True)
            z = ew.tile([FC, NPIX], MMD, tag="z")
            nc.scalar.activation(z[:], ps1[:], AF.Relu, bias=bt1[:])
            for b in range(BL):
                ph = HD * b
                ps2 = psp.tile([FC, NPIX], F32, tag="ps", name="ps2")
                for j in range(2):
                    nc.tensor.matmul(ps2[:, 512 * j:512 * (j + 1)],
                                     wt2[ph:ph + HD, :],
                                     z[ph:ph + HD, 512 * j:512 * (j + 1)],
                                     start=True, stop=True)
                mso = ew.tile([FC, NPIX], F32, tag="mso")
                nc.scalar.activation(mso[0:HD, :], ps2[0:HD, :], AF.Identity,
                                     bias=bt2[0:HD, :])
                nc.scalar.activation(mso[HD:FC, :], ps2[HD:FC, :],
                                     AF.Abs, bias=bt2[HD:FC, :])
                nc.sync.dma_start(mean_d[b], mso[0:HD, :])
                nc.sync.dma_start(std_d[b], mso[HD:FC, :])

    nc.compile()
    return nc


def kernel(input_tensor, time_steps, mask, w_gates, b_gates, w_can, b_can,
           w_ode, b_ode, w_t1, b_t1, w_t2, b_t2):
    global last_result
    input_tensor = np.asarray(input_tensor, np.float32)
    time_steps = np.asarray(time_steps, np.float32)
    mask = np.asarray(mask, np.float32)
    w_gates = np.asarray(w_gates, np.float32)
    w_can = np.asarray(w_can, np.float32)
    w_ode = np.asarray(w_ode, np.float32)

    # host-side prep -------------------------------------------------
    xs = np.ascontiguousarray(
        np.transpose(input_tensor[:, ::-1], (1, 2, 0, 3, 4)))  # (T,C,B,H,W)
    ts_rev = time_steps[::-1].astype(np.float64)
    dts = np.concatenate([[-0.01], ts_rev[1:] - ts_rev[:-1]]).astype(np.float32)
    ms_all = mask[:, ::-1].T.astype(np.float32)      # (T, B)
    use_mask = not np.all(ms_all == 1.0)

    FC = 2 * C
    swap = np.r_[C:FC, 0:C]
    ident = np.arange(FC)

    def lhsT9(w, in_perm, out_perm=None):
        o, i = w.shape[0], w.shape[1]
        out = np.empty((i, 9, o), np.float32)
        for k, (dy, dx) in enumerate(_offsets()):
            m = w[:, :, dy, dx].T[in_perm]
            if out_perm is not None:
                m = m[:, out_perm]
            out[:, k] = m
        return np.ascontiguousarray(out.reshape(i, 9 * o))

    def bdiag9(w):  # (64,64,3,3) -> block-diag (128, 9*128)
        out = np.zeros((FC, 9, FC), np.float32)
        for k, (dy, dx) in enumerate(_offsets()):
            m = w[:, :, dy, dx].T
            out[0:C, k, 0:C] = m
            out[C:FC, k, C:FC] = m
        return np.ascontiguousarray(out.reshape(FC, 9 * FC))

    wg_h = np.stack([lhsT9(w_gates, swap),
                     lhsT9(w_gates, ident, out_perm=swap)])
    wcx_h = bdiag9(w_can[:, 0:C])
    wch_h = bdiag9(w_can[:, C:FC])
    wo_h = np.concatenate([bdiag9(w_ode), bdiag9(-w_ode)], axis=1)
    wt1m = np.asarray(w_t1, np.float32)[:, :, 0, 0].T
    wt1_h = np.zeros((FC, FC), np.float32)
    wt1_h[0:C, 0:C] = wt1m
    wt1_h[C:FC, C:FC] = wt1m
    wt2_h = np.concatenate([np.asarray(w_t2, np.float32)[:, :, 0, 0].T] * 2, 0)

    bgn = np.asarray(b_gates, np.float32)
    bon = np.asarray(b_ode, np.float32)
    dup = lambda v: np.concatenate([v, v]).reshape(-1, 1)

    common = {
        "wg": wg_h, "wcx": wcx_h, "wch": wch_h, "wo": wo_h,
        "wt1": wt1_h, "wt2": wt2_h,
        "bg": np.stack([bgn.reshape(-1, 1), bgn[swap].reshape(-1, 1)]),
        "bc": dup(np.asarray(b_can, np.float32)),
        "bo": np.ascontiguousarray(np.concatenate([dup(bon), dup(-bon)], axis=1)),
        "bt1": dup(np.asarray(b_t1, np.float32)),
        "bt2": np.asarray(b_t2, np.float32).reshape(FC, 1),
    }

    in_maps = []
    for core in range(NCORES):
        bsl = slice(core * BL, (core + 1) * BL)
        m = dict(common)
        m["xs"] = np.ascontiguousarray(xs[:, :, bsl])
        if use_mask:
            mcore = ms_all[:, bsl]
            m["ms"] = np.ascontiguousarray(
                np.broadcast_to(mcore[:, :, None, None], (T, BL, HD, 1))
            ).astype(np.float32)
        in_maps.append(m)

    nc = _build(dts, use_mask)

    trace = bool(int(os.environ.get("KERNEL_TRACE", "0")))
    res = bass_utils.run_bass_kernel_spmd(
        nc, in_maps, core_ids=list(range(NCORES)), trace=trace)
    last_result = res

    mean = np.empty((B, HD, H, W), np.float32)
    std = np.empty((B, HD, H, W), np.float32)
    for core in range(NCORES):
        mean[core * BL:(core + 1) * BL] = res.results[core]["mean"]
        std[core * BL:(core + 1) * BL] = res.results[core]["std"]
    return mean, std


# revision 19
# speedup vs baseline: 1.2264x; 1.0174x over previous
"""Trainium2 Bass kernel for the Encoder-z0 ODE-ConvGRU problem.

Data-parallel over batch: 16 batch elements / 8 NeuronCores = 2 per core.
Per core, a 16-step backwards ConvGRU recurrence with an Euler ODE step,
followed by a 1x1-conv transform producing (mean_z0, std_z0).

Conv3x3 (SAME) is computed as 9 shifted matmuls accumulating in PSUM:
feature maps live in SBUF as zero-padded (34x34) images with channels on
partitions; offset (dy,dx) contributes lhsT[k].T @ shifted_view(rhs).

The two local batch elements are laid out on opposite partition halves
(b=0: 0-63, b=1: 64-127).  All M=64 convolutions (ODE, candidate halves,
first 1x1) are merged across the two batch elements into single full-array
K=128 x M=128 matmuls with block-diagonal weights, halving their PE time.
The candidate conv is further split into an x-part (independent of the
recurrent state - scheduled to fill the recurrence's serial tail) and an
rh-part, accumulating into the same PSUM bank.

dt = -1 steps fold the Euler scale into negated ODE weights (tanh is odd).
Matmuls run in float32r (full-rate PE streaming, ~1e-3 accuracy);
elementwise runs in fp32.
"""

import os

import numpy as np

import concourse.bass as bass
import concourse.tile as tile
from concourse import bacc, mybir
from concourse import bass_utils

B, T, C, H, W = 16, 16, 64, 32, 32
HD = 64
NCORES = 8
BL = B // NCORES          # batch elements per core
P = H + 2                 # padded image edge (34)
NPIX = H * W              # 1024
MMD = mybir.dt.float32r   # matmul dtype
F32 = mybir.dt.float32

last_result = None


def _offsets():
    return [(dy, dx) for dy in range(3) for dx in range(3)]


def _build(dts, use_mask):
    nc = bacc.Bacc("TRN2", target_bir_lowering=False, debug=False,
                   num_devices=NCORES)

    FC = 2 * C  # 128
    xs_d = nc.dram_tensor("xs", [T, C, BL, H, W], MMD, kind="ExternalInput").ap()
    wg_d = nc.dram_tensor("wg", [BL, FC, 9 * FC], MMD, kind="ExternalInput").ap()
    wcx_d = nc.dram_tensor("wcx", [FC, 9 * FC], MMD, kind="ExternalInput").ap()
    wch_d = nc.dram_tensor("wch", [FC, 9 * FC], MMD, kind="ExternalInput").ap()
    wo_d = nc.dram_tensor("wo", [FC, 2 * 9 * FC], MMD, kind="ExternalInput").ap()
    wt1_d = nc.dram_tensor("wt1", [FC, FC], MMD, kind="ExternalInput").ap()
    wt2_d = nc.dram_tensor("wt2", [FC, FC], MMD, kind="ExternalInput").ap()
    bg_d = nc.dram_tensor("bg", [BL, FC, 1], F32, kind="ExternalInput").ap()
    bc_d = nc.dram_tensor("bc", [FC, 1], F32, kind="ExternalInput").ap()
    bo_d = nc.dram_tensor("bo", [FC, 2], F32, kind="ExternalInput").ap()
    bt1_d = nc.dram_tensor("bt1", [FC, 1], F32, kind="ExternalInput").ap()
    bt2_d = nc.dram_tensor("bt2", [FC, 1], F32, kind="ExternalInput").ap()
    if use_mask:
        msd = nc.dram_tensor("ms", [T, BL, HD, 1], F32, kind="ExternalInput").ap()
    mean_d = nc.dram_tensor("mean", [BL, HD, H, W], F32, kind="ExternalOutput").ap()
    std_d = nc.dram_tensor("std", [BL, HD, H, W], F32, kind="ExternalOutput").ap()

    AF = mybir.ActivationFunctionType
    offs = _offsets()

    with tile.TileContext(nc) as tc:
        with (
            tc.tile_pool(name="persist", bufs=1) as pp,
            tc.tile_pool(name="ew", bufs=3) as ew,
            tc.tile_pool(name="psum", bufs=4, space="PSUM") as psp,
        ):
            # ---- persistent state ----
            hbuf = pp.tile([FC, P, P], MMD, name="hbuf")    # h: b0 low, b1 high
            hodebuf = pp.tile([FC, P, P], MMD, name="hodebuf")  # h_ode, both b
            xbuf = pp.tile([FC, P, P], MMD, name="xbuf")    # x: b0 low, b1 high
            rhbuf = pp.tile([FC, P, P], MMD, name="rhbuf")  # r*h_ode per half
            bufa = [pp.tile([FC, P, P], MMD, name=f"bufa{b}") for b in range(BL)]
            wg = [pp.tile([FC, 9 * FC], MMD, name=f"wg{b}") for b in range(BL)]
            wcx = pp.tile([FC, 9 * FC], MMD, name="wcx")
            wch = pp.tile([FC, 9 * FC], MMD, name="wch")
            wo = pp.tile([FC, 2 * 9 * FC], MMD, name="wo")
            wt1 = pp.tile([FC, FC], MMD, name="wt1")
            wt2 = pp.tile([FC, FC], MMD, name="wt2")
            bg = [pp.tile([FC, 1], F32, name=f"bg{b}") for b in range(BL)]
            bc = pp.tile([FC, 1], F32, name="bc")
            bo = pp.tile([FC, 2], F32, name="bo")           # [plain, negated]
            bt1 = pp.tile([FC, 1], F32, name="bt1")
            bt2 = pp.tile([FC, 1], F32, name="bt2")

            nc.vector.memzero(hbuf[:])
            nc.vector.memzero(hodebuf[:])
            nc.vector.memzero(xbuf[:])
            nc.vector.memzero(rhbuf[:])
            for b in range(BL):
                nc.vector.memzero(bufa[b][:])
                nc.sync.dma_start(wg[b][:], wg_d[b])
                nc.sync.dma_start(bg[b][:], bg_d[b])
            nc.sync.dma_start(wcx[:], wcx_d[:])
            nc.sync.dma_start(wch[:], wch_d[:])
            nc.sync.dma_start(wo[:], wo_d[:])
            nc.sync.dma_start(wt1[:], wt1_d[:])
            nc.sync.dma_start(wt2[:], wt2_d[:])
            nc.sync.dma_start(bc[:], bc_d[:])
            nc.sync.dma_start(bo[:], bo_d[:])
            nc.sync.dma_start(bt1[:], bt1_d[:])
            nc.sync.dma_start(bt2[:], bt2_d[:])

            def conv(psum_t, wtile, wcol0, rhs_buf, first, last):
                """Emit 18 full-width conv matmuls (9 offsets x 2 N-halves)."""
                for j in range(2):
                    r0 = 16 * j
                    for k, (dy, dx) in enumerate(offs):
                        nc.tensor.matmul(
                            psum_t[:, 512 * j:512 * (j + 1)],
                            wtile[:, wcol0 + FC * k:wcol0 + FC * (k + 1)],
                            rhs_buf[:, dy + r0:dy + r0 + 16, dx:dx + 32],
                            start=(first and k == 0), stop=(last and k == 8),
                            skip_group_check=True,
                        )

            def intr(buf, p0, pn):
                return buf[p0:p0 + pn, 1:33, 1:33]

            def r3(ap):
                return ap.rearrange("p (y x) -> p y x", y=H, x=W)

            def load_x(t):
                nc.sync.dma_start(intr(xbuf, 0, C), xs_d[t, :, 0])
                nc.sync.dma_start(intr(xbuf, C, C), xs_d[t, :, 1])
                nc.sync.dma_start(intr(bufa[0], C, C), xs_d[t, :, 0])
                nc.sync.dma_start(intr(bufa[1], 0, C), xs_d[t, :, 1])

            load_x(0)

            for t in range(T):
                # --- candidate conv, x part (independent of h) ---
                ps_c = psp.tile([FC, NPIX], F32, tag="ps", name="ps_c")
                conv(ps_c, wcx, 0, xbuf, True, False)

                # --- ODE step, both batch halves (block-diag weights) ---
                wcol = 9 * FC if dts[t] == -1.0 else 0
                ps_o = psp.tile([FC, NPIX], F32, tag="ps", name="ps_o")
                conv(ps_o, wo, wcol, hbuf, True, True)
                neg = 1 if dts[t] == -1.0 else 0
                t1 = ew.tile([FC, NPIX], F32, tag="t1")
                nc.scalar.activation(t1[:], ps_o[:], AF.Tanh,
                                     bias=bo[:, neg:neg + 1])
                src = t1
                if dts[t] not in (1.0, -1.0):
                    tmp = ew.tile([FC, NPIX], F32, tag="tmp")
                    nc.scalar.mul(tmp[:], t1[:], float(dts[t]))
                    src = tmp
                nc.vector.tensor_add(intr(hodebuf, 0, FC),
                                     intr(hbuf, 0, FC), r3(src[:]))
                # gates rhs needs per-b copies of the h_ode halves
                nc.sync.dma_start(intr(bufa[0], 0, HD), intr(hodebuf, 0, HD))
                nc.sync.dma_start(intr(bufa[1], HD, HD), intr(hodebuf, HD, HD))

                # --- gates convs (per b, full K=M=128) + sigmoid, rh, u ---
                u = ew.tile([FC, NPIX], F32, tag="u")
                g = [None, None]
                for b in range(BL):
                    ph, px = HD * b, HD * (1 - b)
                    ps_g = psp.tile([FC, NPIX], F32, tag="ps", name="ps_g")
                    conv(ps_g, wg[b], 0, bufa[b], True, True)
                    gt = ew.tile([FC, NPIX], F32, tag=f"g{b}")
                    nc.scalar.activation(gt[:], ps_g[:], AF.Sigmoid, bias=bg[b][:])
                    g[b] = gt
                    nc.vector.tensor_mul(intr(rhbuf, ph, HD),
                                         r3(gt[ph:ph + HD, :]),
                                         intr(hodebuf, ph, HD))
                    nc.sync.dma_start(u[ph:ph + HD, :], gt[px:px + HD, :])

                # --- candidate conv, rh part (accumulates into ps_c) ---
                conv(ps_c, wch, 0, rhbuf, False, True)

                # --- cand + combine, both halves in single 128p ops ---
                cand = ew.tile([FC, NPIX], F32, tag="cand")
                nc.scalar.activation(cand[:], ps_c[:], AF.Tanh, bias=bc[:])
                d = ew.tile([FC, NPIX], F32, tag="d")
                nc.vector.tensor_sub(r3(d[:]), r3(cand[:]), intr(hodebuf, 0, FC))
                ud = ew.tile([FC, NPIX], F32, tag="ud")
                nc.vector.tensor_mul(ud[:], u[:], d[:])
                uds = ud
                if use_mask:
                    mt = ew.tile([FC, 1], F32, tag="mt")
                    for b in range(BL):
                        nc.sync.dma_start(mt[HD * b:HD * b + HD, :], msd[t, b])
                    ud2 = ew.tile([FC, NPIX], F32, tag="ud2")
                    nc.vector.tensor_single_scalar(
                        ud2[:], ud[:], mt[:], mybir.AluOpType.mult)
                    uds = ud2
                nc.vector.tensor_add(intr(hbuf, 0, FC), intr(hodebuf, 0, FC),
                                     r3(uds[:]))

                if t + 1 < T:
                    load_x(t + 1)

            # ---- transform_z0: conv1x1 -> ReLU -> conv1x1 --------------
            ps1 = psp.tile([FC, NPIX], F32, tag="ps", name="ps1")
            for j in range(2):
                nc.tensor.matmul(ps1[:, 512 * j:512 * (j + 1)], wt1[:],
                                 hbuf[:, 1 + 16 * j:17 + 16 * j, 1:33],
                                 start=True, stop=True)
            z = ew.tile([FC, NPIX], MMD, tag="z")
            nc.scalar.activation(z[:], ps1[:], AF.Relu, bias=bt1[:])
            for b in range(BL):
                ph = HD * b
                ps2 = psp.tile([FC, NPIX], F32, tag="ps", name="ps2")
                for j in range(2):
                    nc.tensor.matmul(ps2[:, 512 * j:512 * (j + 1)],
                                     wt2[ph:ph + HD, :],
                                     z[ph:ph + HD, 512 * j:512 * (j + 1)],
                                     start=True, stop=True)
                mso = ew.tile([FC, NPIX], F32, tag="mso")
                nc.scalar.activation(mso[0:HD, :], ps2[0:HD, :], AF.Identity,
                                     bias=bt2[0:HD, :])
                nc.scalar.activation(mso[HD:FC, :], ps2[HD:FC, :],
                                     AF.Abs, bias=bt2[HD:FC, :])
                nc.sync.dma_start(mean_d[b], mso[0:HD, :])
                nc.sync.dma_start(std_d[b], mso[HD:FC, :])

    nc.compile()
    return nc


def kernel(input_tensor, time_steps, mask, w_gates, b_gates, w_can, b_can,
           w_ode, b_ode, w_t1, b_t1, w_t2, b_t2):
    global last_result
    input_tensor = np.asarray(input_tensor, np.float32)
    time_steps = np.asarray(time_steps, np.float32)
    mask = np.asarray(mask, np.float32)
    w_gates = np.asarray(w_gates, np.float32)
    w_can = np.asarray(w_can, np.float32)
    w_ode = np.asarray(w_ode, np.float32)

    # host-side prep -------------------------------------------------
    xs = np.ascontiguousarray(
        np.transpose(input_tensor[:, ::-1], (1, 2, 0, 3, 4)))  # (T,C,B,H,W)
    ts_rev = time_steps[::-1].astype(np.float64)
    dts = np.concatenate([[-0.01], ts_rev[1:] - ts_rev[:-1]]).astype(np.float32)
    ms_all = mask[:, ::-1].T.astype(np.float32)      # (T, B)
    use_mask = not np.all(ms_all == 1.0)

    FC = 2 * C
    swap = np.r_[C:FC, 0:C]
    ident = np.arange(FC)

    def lhsT9(w, in_perm, out_perm=None):
        o, i = w.shape[0], w.shape[1]
        out = np.empty((i, 9, o), np.float32)
        for k, (dy, dx) in enumerate(_offsets()):
            m = w[:, :, dy, dx].T[in_perm]
            if out_perm is not None:
                m = m[:, out_perm]
            out[:, k] = m
        return np.ascontiguousarray(out.reshape(i, 9 * o))

    def bdiag9(w):  # (64,64,3,3) -> block-diag (128, 9*128)
        out = np.zeros((FC, 9, FC), np.float32)
        for k, (dy, dx) in enumerate(_offsets()):
            m = w[:, :, dy, dx].T
            out[0:C, k, 0:C] = m
            out[C:FC, k, C:FC] = m
        return np.ascontiguousarray(out.reshape(FC, 9 * FC))

    wg_h = np.stack([lhsT9(w_gates, swap),
                     lhsT9(w_gates, ident, out_perm=swap)])
    wcx_h = bdiag9(w_can[:, 0:C])
    wch_h = bdiag9(w_can[:, C:FC])
    wo_h = np.concatenate([bdiag9(w_ode), bdiag9(-w_ode)], axis=1)
    wt1m = np.asarray(w_t1, np.float32)[:, :, 0, 0].T
    wt1_h = np.zeros((FC, FC), np.float32)
    wt1_h[0:C, 0:C] = wt1m
    wt1_h[C:FC, C:FC] = wt1m
    wt2_h = np.concatenate([np.asarray(w_t2, np.float32)[:, :, 0, 0].T] * 2, 0)

    bgn = np.asarray(b_gates, np.float32)
    bon = np.asarray(b_ode, np.float32)
    dup = lambda v: np.concatenate([v, v]).reshape(-1, 1)

    common = {
        "wg": wg_h, "wcx": wcx_h, "wch": wch_h, "wo": wo_h,
        "wt1": wt1_h, "wt2": wt2_h,
        "bg": np.stack([bgn.reshape(-1, 1), bgn[swap].reshape(-1, 1)]),
        "bc": dup(np.asarray(b_can, np.float32)),
        "bo": np.ascontiguousarray(np.concatenate([dup(bon), dup(-bon)], axis=1)),
        "bt1": dup(np.asarray(b_t1, np.float32)),
        "bt2": np.asarray(b_t2, np.float32).reshape(FC, 1),
    }

    in_maps = []
    for core in range(NCORES):
        bsl = slice(core * BL, (core + 1) * BL)
        m = dict(common)
        m["xs"] = np.ascontiguousarray(xs[:, :, bsl])
        if use_mask:
            mcore = ms_all[:, bsl]
            m["ms"] = np.ascontiguousarray(
                np.broadcast_to(mcore[:, :, None, None], (T, BL, HD, 1))
            ).astype(np.float32)
        in_maps.append(m)

    nc = _build(dts, use_mask)

    trace = bool(int(os.environ.get("KERNEL_TRACE", "0")))
    res = bass_utils.run_bass_kernel_spmd(
        nc, in_maps, core_ids=list(range(NCORES)), trace=trace)
    last_result = res

    mean = np.empty((B, HD, H, W), np.float32)
    std = np.empty((B, HD, H, W), np.float32)
    for core in range(NCORES):
        mean[core * BL:(core + 1) * BL] = res.results[core]["mean"]
        std[core * BL:(core + 1) * BL] = res.results[core]["std"]
    return mean, std


# revision 22
# speedup vs baseline: 1.3381x; 1.0911x over previous
"""Trainium2 Bass kernel for the Encoder-z0 ODE-ConvGRU problem.

Data-parallel over batch: 16 batch elements / 8 NeuronCores = 2 per core.
Per core, a 16-step backwards ConvGRU recurrence with an Euler ODE step,
followed by a 1x1-conv transform producing (mean_z0, std_z0).

Conv3x3 (SAME) is computed as 9 shifted matmuls accumulating in PSUM:
feature maps live in SBUF as zero-padded (34x34) images with channels on
partitions; offset (dy,dx) contributes lhsT[k].T @ shifted_view(rhs).

The two local batch elements are laid out on opposite partition halves
(b=0: 0-63, b=1: 64-127).  All M=64 convolutions (ODE, candidate halves,
first 1x1) are merged across the two batch elements into single full-array
K=128 x M=128 matmuls with block-diagonal weights, halving their PE time.
The candidate conv is further split into an x-part (independent of the
recurrent state - scheduled to fill the recurrence's serial tail) and an
rh-part, accumulating into the same PSUM bank.

dt = -1 steps fold the Euler scale into negated ODE weights (tanh is odd).
Matmuls run in float32r (full-rate PE streaming, ~1e-3 accuracy);
elementwise runs in fp32.
"""

import os

import numpy as np

import concourse.bass as bass
import concourse.tile as tile
from concourse import bacc, mybir
from concourse import bass_utils

B, T, C, H, W = 16, 16, 64, 32, 32
HD = 64
NCORES = 8
BL = B // NCORES          # batch elements per core
P = H + 2                 # padded image edge (34)
NPIX = H * W              # 1024
MMD = mybir.dt.float32r   # matmul dtype
F32 = mybir.dt.float32

last_result = None


def _offsets():
    return [(dy, dx) for dy in range(3) for dx in range(3)]


def _build(dts, use_mask):
    nc = bacc.Bacc("TRN2", target_bir_lowering=False, debug=False,
                   num_devices=NCORES)

    FC = 2 * C  # 128
    xs_d = nc.dram_tensor("xs", [T, C, BL, H, W], MMD, kind="ExternalInput").ap()
    wg_d = nc.dram_tensor("wg", [BL, FC, 9 * FC], MMD, kind="ExternalInput").ap()
    wcx_d = nc.dram_tensor("wcx", [FC, 9 * FC], MMD, kind="ExternalInput").ap()
    wch_d = nc.dram_tensor("wch", [FC, 9 * FC], MMD, kind="ExternalInput").ap()
    wo_d = nc.dram_tensor("wo", [FC, 2 * 9 * FC], MMD, kind="ExternalInput").ap()
    wt1_d = nc.dram_tensor("wt1", [FC, FC], MMD, kind="ExternalInput").ap()
    wt2_d = nc.dram_tensor("wt2", [FC, FC], MMD, kind="ExternalInput").ap()
    bg_d = nc.dram_tensor("bg", [BL, FC, 1], F32, kind="ExternalInput").ap()
    bc_d = nc.dram_tensor("bc", [FC, 1], F32, kind="ExternalInput").ap()
    bo_d = nc.dram_tensor("bo", [FC, 2], F32, kind="ExternalInput").ap()
    bt1_d = nc.dram_tensor("bt1", [FC, 1], F32, kind="ExternalInput").ap()
    bt2_d = nc.dram_tensor("bt2", [FC, 1], F32, kind="ExternalInput").ap()
    if use_mask:
        msd = nc.dram_tensor("ms", [T, BL, HD, 1], F32, kind="ExternalInput").ap()
    mean_d = nc.dram_tensor("mean", [BL, HD, H, W], F32, kind="ExternalOutput").ap()
    std_d = nc.dram_tensor("std", [BL, HD, H, W], F32, kind="ExternalOutput").ap()

    AF = mybir.ActivationFunctionType
    offs = _offsets()

    with tile.TileContext(nc) as tc:
        with (
            tc.tile_pool(name="persist", bufs=1) as pp,
            tc.tile_pool(name="ew", bufs=3) as ew,
            tc.tile_pool(name="psum", bufs=4, space="PSUM") as psp,
        ):
            # ---- persistent state ----
            hbuf = pp.tile([FC, P, P], MMD, name="hbuf")    # h: b0 low, b1 high
            hodebuf = pp.tile([FC, P, P], MMD, name="hodebuf")  # h_ode, both b
            xbuf = pp.tile([FC, P, P], MMD, name="xbuf")    # x: b0 low, b1 high
            rhbuf = pp.tile([FC, P, P], MMD, name="rhbuf")  # r*h_ode per half
            bufa = [pp.tile([FC, P, P], MMD, name=f"bufa{b}") for b in range(BL)]
            wg = [pp.tile([FC, 9 * FC], MMD, name=f"wg{b}") for b in range(BL)]
            wcx = pp.tile([FC, 9 * FC], MMD, name="wcx")
            wch = pp.tile([FC, 9 * FC], MMD, name="wch")
            wo = pp.tile([FC, 2 * 9 * FC], MMD, name="wo")
            wt1 = pp.tile([FC, FC], MMD, name="wt1")
            wt2 = pp.tile([FC, FC], MMD, name="wt2")
            bg = [pp.tile([FC, 1], F32, name=f"bg{b}") for b in range(BL)]
            bc = pp.tile([FC, 1], F32, name="bc")
            bo = pp.tile([FC, 2], F32, name="bo")           # [plain, negated]
            bt1 = pp.tile([FC, 1], F32, name="bt1")
            bt2 = pp.tile([FC, 1], F32, name="bt2")

            nc.vector.memzero(hbuf[:])
            nc.vector.memzero(hodebuf[:])
            nc.vector.memzero(xbuf[:])
            nc.vector.memzero(rhbuf[:])
            for b in range(BL):
                nc.vector.memzero(bufa[b][:])
                nc.sync.dma_start(wg[b][:], wg_d[b])
                nc.sync.dma_start(bg[b][:], bg_d[b])
            nc.sync.dma_start(wcx[:], wcx_d[:])
            nc.sync.dma_start(wch[:], wch_d[:])
            nc.sync.dma_start(wo[:], wo_d[:])
            nc.sync.dma_start(wt1[:], wt1_d[:])
            nc.sync.dma_start(wt2[:], wt2_d[:])
            nc.sync.dma_start(bc[:], bc_d[:])
            nc.sync.dma_start(bo[:], bo_d[:])
            nc.sync.dma_start(bt1[:], bt1_d[:])
            nc.sync.dma_start(bt2[:], bt2_d[:])

            def convj(psum_t, wtile, wcol0, rhs_buf, j, first, last):
                """Emit the 9 conv matmuls for output-row half j."""
                r0 = 16 * j
                for k, (dy, dx) in enumerate(offs):
                    nc.tensor.matmul(
                        psum_t[:, 512 * j:512 * (j + 1)],
                        wtile[:, wcol0 + FC * k:wcol0 + FC * (k + 1)],
                        rhs_buf[:, dy + r0:dy + r0 + 16, dx:dx + 32],
                        start=(first and k == 0), stop=(last and k == 8),
                        skip_group_check=True,
                    )

            def conv(psum_t, wtile, wcol0, rhs_buf, first, last):
                convj(psum_t, wtile, wcol0, rhs_buf, 0, first, last)
                convj(psum_t, wtile, wcol0, rhs_buf, 1, first, last)

            def intr(buf, p0, pn):
                return buf[p0:p0 + pn, 1:33, 1:33]

            def intrr(buf, p0, pn, r0, rn):
                return buf[p0:p0 + pn, 1 + r0:1 + r0 + rn, 1:33]

            def r3(ap):
                return ap.rearrange("p (y x) -> p y x", y=H, x=W)

            def r3r(ap, r0, rn):
                return r3(ap)[:, r0:r0 + rn, :]

            # pixel-row chunks: A covers the j=0 conv reads (rows 0-17 incl
            # halo), B the rest.
            RCH = ((0, 18), (18, 14))

            def load_x(t):
                nc.sync.dma_start(intr(xbuf, 0, C), xs_d[t, :, 0])
                nc.sync.dma_start(intr(xbuf, C, C), xs_d[t, :, 1])
                nc.sync.dma_start(intr(bufa[0], C, C), xs_d[t, :, 0])
                nc.sync.dma_start(intr(bufa[1], 0, C), xs_d[t, :, 1])

            load_x(0)

            for t in range(T):
                # h_next = (1-u)*h_ode + u*cand: the (1-u)*h_ode term and all
                # non-tail work is scheduled off the critical path; the ops
                # feeding the next conv block are chunked so the first row
                # block (A) is ready while the PE streams filler matmuls.

                # canx j0: fills the previous step's tail
                ps_c = psp.tile([FC, NPIX], F32, tag="ps", name="ps_c")
                convj(ps_c, wcx, 0, xbuf, 0, True, False)

                # ODE conv, both batch halves (block-diag weights)
                wcol = 9 * FC if dts[t] == -1.0 else 0
                neg = 1 if dts[t] == -1.0 else 0
                ps_o = psp.tile([FC, NPIX], F32, tag="ps", name="ps_o")
                convj(ps_o, wo, wcol, hbuf, 0, True, True)
                convj(ps_o, wo, wcol, hbuf, 1, True, True)
                t1 = ew.tile([FC, NPIX], F32, tag="t1")
                nc.scalar.activation(t1[:, 0:512], ps_o[:, 0:512], AF.Tanh,
                                     bias=bo[:, neg:neg + 1])
                nc.scalar.activation(t1[:, 512:NPIX], ps_o[:, 512:NPIX],
                                     AF.Tanh, bias=bo[:, neg:neg + 1])
                src = t1
                if dts[t] not in (1.0, -1.0):
                    tmp = ew.tile([FC, NPIX], F32, tag="tmp")
                    nc.scalar.mul(tmp[:], t1[:], float(dts[t]))
                    src = tmp

                # canx j1: fills the h_ode chain
                convj(ps_c, wcx, 0, xbuf, 1, True, False)

                # h_ode (chunked) + per-b copies of its halves for gates rhs
                for r0, rn in RCH:
                    nc.vector.tensor_add(intrr(hodebuf, 0, FC, r0, rn),
                                         intrr(hbuf, 0, FC, r0, rn),
                                         r3r(src[:], r0, rn))
                    nc.sync.dma_start(intrr(bufa[0], 0, HD, r0, rn),
                                      intrr(hodebuf, 0, HD, r0, rn))
                    nc.sync.dma_start(intrr(bufa[1], HD, HD, r0, rn),
                                      intrr(hodebuf, HD, HD, r0, rn))

                # gates convs (per b, full K=M=128) + sigmoid, rh, u
                u = ew.tile([FC, NPIX], F32, tag="u")
                for b in range(BL):
                    ph, px = HD * b, HD * (1 - b)
                    ps_g = psp.tile([FC, NPIX], F32, tag="ps", name="ps_g")
                    convj(ps_g, wg[b], 0, bufa[b], 0, True, True)
                    convj(ps_g, wg[b], 0, bufa[b], 1, True, True)
                    gt = ew.tile([FC, NPIX], F32, tag=f"g{b}")
                    nc.scalar.activation(gt[:, 0:512], ps_g[:, 0:512],
                                         AF.Sigmoid, bias=bg[b][:])
                    nc.scalar.activation(gt[:, 512:NPIX], ps_g[:, 512:NPIX],
                                         AF.Sigmoid, bias=bg[b][:])
                    for r0, rn in RCH:
                        nc.vector.tensor_mul(intrr(rhbuf, ph, HD, r0, rn),
                                             r3r(gt[ph:ph + HD, :], r0, rn),
                                             intrr(hodebuf, ph, HD, r0, rn))
                    nc.sync.dma_start(u[ph:ph + HD, :], gt[px:px + HD, :])

                # off-critical-path tail prep: u' = m*u, om = 1-u', f = om*h_ode
                us = u
                if use_mask:
                    mt = ew.tile([FC, 1], F32, tag="mt")
                    for b in range(BL):
                        nc.sync.dma_start(mt[HD * b:HD * b + HD, :], msd[t, b])
                    u2 = ew.tile([FC, NPIX], F32, tag="u2")
                    nc.vector.tensor_single_scalar(
                        u2[:], u[:], mt[:], mybir.AluOpType.mult)
                    us = u2
                om = ew.tile([FC, NPIX], F32, tag="om")
                nc.vector.tensor_scalar(om[:], us[:], -1.0, 1.0,
                                        mybir.AluOpType.mult,
                                        mybir.AluOpType.add)
                f = ew.tile([FC, NPIX], F32, tag="f")
                nc.vector.tensor_mul(r3(f[:]), r3(om[:]), intr(hodebuf, 0, FC))

                # candidate conv, rh part (accumulates into ps_c)
                convj(ps_c, wch, 0, rhbuf, 0, False, True)
                convj(ps_c, wch, 0, rhbuf, 1, False, True)

                # tail: cand (chunked by bank), then h_next chunked by rows
                cand = ew.tile([FC, NPIX], F32, tag="cand")
                nc.scalar.activation(cand[:, 0:512], ps_c[:, 0:512], AF.Tanh,
                                     bias=bc[:])
                nc.scalar.activation(cand[:, 512:NPIX], ps_c[:, 512:NPIX],
                                     AF.Tanh, bias=bc[:])
                e = ew.tile([FC, NPIX], F32, tag="e")
                for r0, rn in RCH:
                    nc.vector.tensor_mul(r3r(e[:], r0, rn),
                                         r3r(us[:], r0, rn),
                                         r3r(cand[:], r0, rn))
                    nc.vector.tensor_add(intrr(hbuf, 0, FC, r0, rn),
                                         r3r(f[:], r0, rn),
                                         r3r(e[:], r0, rn))

                if t + 1 < T:
                    load_x(t + 1)

            # ---- transform_z0: conv1x1 -> ReLU -> conv1x1 --------------
            ps1 = psp.tile([FC, NPIX], F32, tag="ps", name="ps1")
            for j in range(2):
                nc.tensor.matmul(ps1[:, 512 * j:512 * (j + 1)], wt1[:],
                                 hbuf[:, 1 + 16 * j:17 + 16 * j, 1:33],
                                 start=True, stop=True)
            z = ew.tile([FC, NPIX], MMD, tag="z")
            nc.scalar.activation(z[:], ps1[:], AF.Relu, bias=bt1[:])
            for b in range(BL):
                ph = HD * b
                ps2 = psp.tile([FC, NPIX], F32, tag="ps", name="ps2")
                for j in range(2):
                    nc.tensor.matmul(ps2[:, 512 * j:512 * (j + 1)],
                                     wt2[ph:ph + HD, :],
                                     z[ph:ph + HD, 512 * j:512 * (j + 1)],
                                     start=True, stop=True)
                mso = ew.tile([FC, NPIX], F32, tag="mso")
                nc.scalar.activation(mso[0:HD, :], ps2[0:HD, :], AF.Identity,
                                     bias=bt2[0:HD, :])
                nc.scalar.activation(mso[HD:FC, :], ps2[HD:FC, :],
                                     AF.Abs, bias=bt2[HD:FC, :])
                nc.sync.dma_start(mean_d[b], mso[0:HD, :])
                nc.sync.dma_start(std_d[b], mso[HD:FC, :])

    nc.compile()
    return nc


def kernel(input_tensor, time_steps, mask, w_gates, b_gates, w_can, b_can,
           w_ode, b_ode, w_t1, b_t1, w_t2, b_t2):
    global last_result
    input_tensor = np.asarray(input_tensor, np.float32)
    time_steps = np.asarray(time_steps, np.float32)
    mask = np.asarray(mask, np.float32)
    w_gates = np.asarray(w_gates, np.float32)
    w_can = np.asarray(w_can, np.float32)
    w_ode = np.asarray(w_ode, np.float32)

    # host-side prep -------------------------------------------------
    xs = np.ascontiguousarray(
        np.transpose(input_tensor[:, ::-1], (1, 2, 0, 3, 4)))  # (T,C,B,H,W)
    ts_rev = time_steps[::-1].astype(np.float64)
    dts = np.concatenate([[-0.01], ts_rev[1:] - ts_rev[:-1]]).astype(np.float32)
    ms_all = mask[:, ::-1].T.astype(np.float32)      # (T, B)
    use_mask = not np.all(ms_all == 1.0)

    FC = 2 * C
    swap = np.r_[C:FC, 0:C]
    ident = np.arange(FC)

    def lhsT9(w, in_perm, out_perm=None):
        o, i = w.shape[0], w.shape[1]
        out = np.empty((i, 9, o), np.float32)
        for k, (dy, dx) in enumerate(_offsets()):
            m = w[:, :, dy, dx].T[in_perm]
            if out_perm is not None:
                m = m[:, out_perm]
            out[:, k] = m
        return np.ascontiguousarray(out.reshape(i, 9 * o))

    def bdiag9(w):  # (64,64,3,3) -> block-diag (128, 9*128)
        out = np.zeros((FC, 9, FC), np.float32)
        for k, (dy, dx) in enumerate(_offsets()):
            m = w[:, :, dy, dx].T
            out[0:C, k, 0:C] = m
            out[C:FC, k, C:FC] = m
        return np.ascontiguousarray(out.reshape(FC, 9 * FC))

    wg_h = np.stack([lhsT9(w_gates, swap),
                     lhsT9(w_gates, ident, out_perm=swap)])
    wcx_h = bdiag9(w_can[:, 0:C])
    wch_h = bdiag9(w_can[:, C:FC])
    wo_h = np.concatenate([bdiag9(w_ode), bdiag9(-w_ode)], axis=1)
    wt1m = np.asarray(w_t1, np.float32)[:, :, 0, 0].T
    wt1_h = np.zeros((FC, FC), np.float32)
    wt1_h[0:C, 0:C] = wt1m
    wt1_h[C:FC, C:FC] = wt1m
    wt2_h = np.concatenate([np.asarray(w_t2, np.float32)[:, :, 0, 0].T] * 2, 0)

    bgn = np.asarray(b_gates, np.float32)
    bon = np.asarray(b_ode, np.float32)
    dup = lambda v: np.concatenate([v, v]).reshape(-1, 1)

    common = {
        "wg": wg_h, "wcx": wcx_h, "wch": wch_h, "wo": wo_h,
        "wt1": wt1_h, "wt2": wt2_h,
        "bg": np.stack([bgn.reshape(-1, 1), bgn[swap].reshape(-1, 1)]),
        "bc": dup(np.asarray(b_can, np.float32)),
        "bo": np.ascontiguousarray(np.concatenate([dup(bon), dup(-bon)], axis=1)),
        "bt1": dup(np.asarray(b_t1, np.float32)),
        "bt2": np.asarray(b_t2, np.float32).reshape(FC, 1),
    }

    in_maps = []
    for core in range(NCORES):
        bsl = slice(core * BL, (core + 1) * BL)
        m = dict(common)
        m["xs"] = np.ascontiguousarray(xs[:, :, bsl])
        if use_mask:
            mcore = ms_all[:, bsl]
            m["ms"] = np.ascontiguousarray(
                np.broadcast_to(mcore[:, :, None, None], (T, BL, HD, 1))
            ).astype(np.float32)
        in_maps.append(m)

    nc = _build(dts, use_mask)

    trace = bool(int(os.environ.get("KERNEL_TRACE", "0")))
    res = bass_utils.run_bass_kernel_spmd(
        nc, in_maps, core_ids=list(range(NCORES)), trace=trace)
    last_result = res

    mean = np.empty((B, HD, H, W), np.float32)
    std = np.empty((B, HD, H, W), np.float32)
    for core in range(NCORES):
        mean[core * BL:(core + 1) * BL] = res.results[core]["mean"]
        std[core * BL:(core + 1) * BL] = res.results[core]["std"]
    return mean, std
